# revision 25
# baseline (speedup 1.0000x reference)
"""HyperGNN message-passing kernel (nn_Conv_13778255086166) for 8 TRN2 NeuronCores.

Reference computation:
    Xp    = X @ W                                   [N, 64]
    Xe_s  = segment_sum(Xp[vertex], edges, E);  cnt = segment_sum(1, edges, E)
    Ze    = (homo / max(cnt,1)) * Xe_s              [E, 64]   (mean aggregation * homo)
    att_s = segment_sum(homo[edges], vertex, N)
    Xv    = segment_sum(Ze[edges], vertex, N) / att_s
    out   = row_l2_normalize(Xp + Xv)

Distribution (graph parallelism): incidences sharded by vertex range; core k
owns nodes [k*12500, (k+1)*12500).  Per core:

  phase 0: Xp = X_local @ W -> fp32 XpD [npcp, 64] (finalize) and a bf16
           gather table XpB [npcp, 128] (cols 0:64 features, col 64 = count
           marker 1.0, pad rows 0.0, rest zero).
  phase 1: per 128-edge tile, dma_gather the XpB rows of the (host-sorted,
           tail-padded) incidence slots and accumulate per 128-slot block
           onehot.T @ g[:, j, 0:65] into PSUM -> [sums | cnt].  The one-hot
           selection matrices are HOST-PRECOMPUTED (pure index formatting)
           and streamed in as bf16 — nothing builds them on-device, which
           keeps the Vector engine out of the Q7 descriptor-generation
           windows (they interfere on the shared SBUF port).  Gather pads
           carry index -1 (the Q7 ucode trims trailing negatives; num_idxs
           per tile = shared max block count so the trimmed count matches
           the decode-side ring accounting on every core).  Gathers
           round-robin over 4 SWDGE queues; their num_idxs registers are
           preloaded once so the GPSIMD queue holds gathers back-to-back.
  AllReduce(Eacc bf16) in chunks; each chunk's reduce and Ze build overlap
           the remaining phase-1 tiles.
  Ze build: ZeF[:, 0:64] = Ered * homo / max(cnt, 1); ZeF[:, 64] = homo.
  phase 2: per 128-node tile, same gather + one-hot matmul against ZeF ->
           PSUM [sum Ze | att_sum]; finalize Xv = S / max(att, eps);
           out = (Xp + Xv) / rownorm (Square/rowsum/scales on the idle ACT
           engine).

All arithmetic (matmul, all segment sums, normalizations) runs on device.
The host only reorganizes the incidence lists (shard by vertex range, order
by segment, pad to fixed per-tile capacity) and formats index/selection
tensors — schedule/layout preparation, not computation.
"""

import os
from dataclasses import dataclass

import ml_dtypes
import numpy as np

import concourse.bacc as bacc
import concourse.mybir as mybir
import concourse.tile as tile
from concourse import bass_utils

F32 = mybir.dt.float32
BF16 = mybir.dt.bfloat16
I16 = mybir.dt.int16
BF = ml_dtypes.bfloat16


@dataclass(frozen=True)
class Cfg:
    n_cores: int = 8
    N: int = 100000
    E: int = 25000
    cap1: int = 1536   # incidence slots per 128-edge tile per core (mult of 128)
    cap2: int = 2944   # incidence slots per 128-node tile per core (mult of 128)
    nqueues: int = 4   # SWDGE queues used for dma_gather round-robin
    ar_chunks: int = 4  # AllReduce chunks overlapped with phase 1
    grp: int = 1       # gathers hoisted per group (>1 corrupts: decode race)
    m1: int = 4        # phase-1 edge tiles merged per dma_gather
    trim_pads: bool = True  # -1 gather pads (Q7 trims trailing negatives)

    @property
    def npc(self):
        assert self.N % self.n_cores == 0
        return self.N // self.n_cores

    @property
    def npcp(self):  # padded, with at least one spare zero row
        return (self.npc + 1 + 127) // 128 * 128

    @property
    def ntiles(self):
        return self.npcp // 128

    @property
    def ep(self):
        return (self.E + 1 + 127) // 128 * 128

    @property
    def etiles(self):
        return self.ep // 128


def wrap_idx(idx: np.ndarray) -> np.ndarray:
    """int16 index layout for dma_gather: element j at [j%16, j//16],
    replicated across the 8 16-partition groups (one per Q7 cpu)."""
    s = idx.shape[0]
    assert s % 16 == 0
    w = np.ascontiguousarray(idx.astype(np.int16).reshape(-1, 16).T)
    return np.tile(w, (8, 1))


def prep_inputs(cfg: Cfg, X, W, homo, vertex, edges):
    """Host-side shard/sort/pad for all cores (index/layout reorganization
    only).  Returns (in_maps, nb1, nb2) where nb1/nb2 are the per-tile
    128-slot block counts shared across cores (max over cores)."""
    npc, npcp = cfg.npc, cfg.npcp
    vertex = np.asarray(vertex)
    edges = np.asarray(edges)

    def build_core(k):
        sel = (vertex >= k * npc) & (vertex < (k + 1) * npc)
        v_l = (vertex[sel] - k * npc).astype(np.int64)
        e_l = edges[sel].astype(np.int64)
        return v_l, e_l

    shards = [build_core(k) for k in range(cfg.n_cores)]

    def sort_one(seg, other, tiles_n, cap):
        o = np.argsort(seg, kind="stable")
        s, g = seg[o], other[o]
        t_of = s >> 7
        counts = np.bincount(t_of, minlength=tiles_n)
        assert (counts <= cap).all(), (counts.max(), cap)
        return s, g, t_of, counts

    sorted1 = [sort_one(e_l, v_l, cfg.etiles, cfg.cap1) for v_l, e_l in shards]
    sorted2 = [sort_one(v_l, e_l, cfg.ntiles, cfg.cap2) for v_l, e_l in shards]

    # shared per-tile block counts: max over cores, >= 1
    nb1 = np.maximum(1, -(-np.stack([c for _, _, _, c in sorted1]).max(0) // 128))
    nb2 = np.maximum(1, -(-np.stack([c for _, _, _, c in sorted2]).max(0) // 128))

    def build(sorted_sg, nb, cap, pad_gather, M):
        """Supertiles of M consecutive 128-seg tiles share one gather: each
        sub-tile's live span [boff, boff + nb*128) is packed back-to-back so
        the supertile's trimmed index count is sum(nb)*128."""
        s, g, t_of, counts = sorted_sg
        tiles_n = len(counts)
        assert tiles_n % M == 0
        gtiles = tiles_n // M
        capg = cap * M
        starts = np.cumsum(counts) - counts
        rank = np.arange(len(s)) - starts[t_of]
        if not cfg.trim_pads:
            assert M == 1
            boff = np.zeros(tiles_n, np.int64)
            nbg = np.full(gtiles, cap // 128 * M, np.int64)
        else:
            nbr = nb.reshape(gtiles, M)
            boff = (np.cumsum(nbr, 1) - nbr).reshape(tiles_n) * 128
            nbg = nbr.sum(1)
        g_of = t_of // M
        dest = g_of * capg + boff[t_of] + rank
        S = gtiles * capg
        gi = np.full(S, -1, np.int64)
        off = np.full(S, 255, np.int64)
        ar = np.arange(capg)
        live = ar[None, :] < (nbg * 128)[:, None]
        gi[live.ravel()] = pad_gather
        gi[dest] = g
        off[dest] = s & 127
        # one-hot selection matrices [gtiles, 128 slot-partitions, blocks*128]
        oh = (off.reshape(gtiles, capg // 128, 128)[:, :, :, None]
              == np.arange(128)[None, None, None, :])
        oh = np.ascontiguousarray(
            oh.transpose(0, 2, 1, 3).reshape(gtiles, 128, capg)
        ).astype(BF)
        return gi, oh, tuple(int(x) for x in nbg)

    def tilemaj_idx(gi, tiles_n, cap):
        w = np.stack([wrap_idx(gi[t * cap:(t + 1) * cap]) for t in range(tiles_n)])
        return np.ascontiguousarray(w)

    homo_pad = np.zeros(cfg.ep, np.float32)
    homo_pad[:cfg.E] = np.asarray(homo)
    homo_t = np.ascontiguousarray(homo_pad.reshape(cfg.etiles, 128).T)

    mark = (np.arange(npcp).reshape(cfg.ntiles, 128).T < npc).astype(np.float32)
    mark = np.ascontiguousarray(mark)

    in_maps = []
    for k in range(cfg.n_cores):
        g1, oh1, nbg1 = build(sorted1[k], nb1, cfg.cap1, npc, cfg.m1)
        g2, oh2, nbg2 = build(sorted2[k], nb2, cfg.cap2, cfg.E, 1)
        Xt = np.zeros((64, npcp), np.float32)
        Xt[:, :npc] = np.asarray(X)[k * npc:(k + 1) * npc].T
        in_maps.append({
            "Xt": Xt,
            "W": np.asarray(W, dtype=np.float32),
            "homo_t": homo_t,
            "mark": mark,
            "g1": tilemaj_idx(g1, cfg.etiles // cfg.m1, cfg.cap1 * cfg.m1),
            "oh1": oh1,
            "g2": tilemaj_idx(g2, cfg.ntiles, cfg.cap2),
            "oh2": oh2,
        })
    return (in_maps, tuple(int(x) for x in nb1), tuple(int(x) for x in nb2),
            nbg1, nbg2)


def build_nc(cfg: Cfg, nb1, nb2):
    # phase-1 supertile block counts/bases (shared across cores)
    M1 = cfg.m1
    nb1r = np.array(nb1).reshape(-1, M1)
    nbg1 = nb1r.sum(1)
    boff1 = (np.cumsum(nb1r, 1) - nb1r)
    c1 = cfg.cap1 // 128
    c2 = cfg.cap2 // 128
    nc = bacc.Bacc("TRN2", target_bir_lowering=False, debug=False,
                   num_devices=cfg.n_cores, num_swdge_queues=cfg.nqueues)

    xt_d = nc.dram_tensor("Xt", [64, cfg.npcp], F32, kind="ExternalInput")
    w_d = nc.dram_tensor("W", [64, 64], F32, kind="ExternalInput")
    homo_d = nc.dram_tensor("homo_t", [128, cfg.etiles], F32, kind="ExternalInput")
    gt1 = cfg.etiles // cfg.m1
    capg1 = cfg.cap1 * cfg.m1
    g1_d = nc.dram_tensor("g1", [gt1, 128, capg1 // 16], I16, kind="ExternalInput")
    oh1_d = nc.dram_tensor("oh1", [gt1, 128, capg1], BF16, kind="ExternalInput")
    g2_d = nc.dram_tensor("g2", [cfg.ntiles, 128, cfg.cap2 // 16], I16, kind="ExternalInput")
    oh2_d = nc.dram_tensor("oh2", [cfg.ntiles, 128, cfg.cap2], BF16, kind="ExternalInput")
    mark_d = nc.dram_tensor("mark", [128, cfg.ntiles], F32, kind="ExternalInput")
    out_d = nc.dram_tensor("out", [cfg.npcp, 64], F32, kind="ExternalOutput")

    xp_d = nc.dram_tensor("XpD", [cfg.npcp, 64], F32, kind="Internal")
    xpb_d = nc.dram_tensor("XpB", [cfg.npcp, 128], BF16, kind="Internal")
    eacc_d = nc.dram_tensor("EaccD", [cfg.ep, 65], BF16, kind="Internal")
    ered_d = nc.dram_tensor("EredD", [cfg.ep, 65], BF16, kind="Internal", addr_space="Shared")
    zef_d = nc.dram_tensor("ZeFD", [cfg.ep, 128], BF16, kind="Internal")

    qn = [0]

    def next_q():
        q = qn[0]
        qn[0] = (q + 1) % cfg.nqueues
        return q

    # phase-1 AllReduce chunk boundaries (tile index); last chunk smallest so
    # the un-overlapped tail is short
    if cfg.ar_chunks == 4:
        cuts = [0, 64, 112, 160, cfg.etiles]
    elif cfg.ar_chunks == 2:
        cuts = [0, cfg.etiles // 2, cfg.etiles]
    else:
        cuts = [0, cfg.etiles]

    with tile.TileContext(nc) as tc:
        with (
            tc.tile_pool(name="const", bufs=1) as pc,
            tc.tile_pool(name="idx", bufs=8) as pidx,
            tc.tile_pool(name="gather", bufs=6) as pg,
            tc.tile_pool(name="onehot", bufs=4) as pm,
            tc.tile_pool(name="sbout", bufs=4) as po,
            tc.tile_pool(name="fin", bufs=4) as pf,
            tc.tile_pool(name="psum", bufs=2, space="PSUM") as pp,
        ):
            # num_idxs registers, preloaded once per distinct value
            nregs = {}
            for v in sorted({int(n) * 128 for n in (*nbg1, *nb2)}):
                r = nc.gpsimd.alloc_register(f"nidx_{v}")
                nc.gpsimd.reg_mov(r, v)
                nregs[v] = r

            w_sb = pc.tile([64, 64], F32)
            nc.sync.dma_start(out=w_sb[:], in_=w_d[:])
            homo_sb = pc.tile([128, cfg.etiles], F32)
            nc.sync.dma_start(out=homo_sb[:], in_=homo_d[:])
            mark_sb = pc.tile([128, cfg.ntiles], F32)
            nc.sync.dma_start(out=mark_sb[:], in_=mark_d[:])

            # phase 0: Xp = X_local @ W; fp32 table + bf16 gather table
            for t in range(cfg.ntiles):
                xtt = pf.tile([64, 128], F32, tag="xt0")
                nc.sync.dma_start(out=xtt[:], in_=xt_d[:, t * 128:(t + 1) * 128])
                ps = pp.tile([128, 64], F32, tag="ps0")
                nc.tensor.matmul(ps[:], lhsT=xtt[:],
                                 rhs=w_sb[:], start=True, stop=True)
                xp_sb = po.tile([128, 64], F32, tag="xp0")
                nc.vector.tensor_copy(out=xp_sb[:], in_=ps[:])
                nc.scalar.dma_start(out=xp_d[t * 128:(t + 1) * 128, :], in_=xp_sb[:])
                xb = po.tile([128, 128], BF16, tag="xb0")
                nc.vector.memset(xb[:, 65:128], 0.0)
                # count marker: 1.0 for real rows, 0.0 for pad/zero rows
                nc.vector.tensor_copy(out=xb[:, 64:65], in_=mark_sb[:, t:t + 1])
                nc.vector.tensor_copy(out=xb[:, 0:64], in_=ps[:])
                nc.sync.dma_start(out=xpb_d[t * 128:(t + 1) * 128, :], in_=xb[:])

            # Ze build for one edge tile (runs as soon as its AR chunk lands)
            def ze_build(t):
                er = pf.tile([128, 65], BF16, tag="er")
                nc.sync.dma_start(out=er[:], in_=ered_d[t * 128:(t + 1) * 128, :])
                cntm = pf.tile([128, 1], F32, tag="cntm")
                nc.vector.tensor_scalar_max(out=cntm[:], in0=er[:, 64:65], scalar1=1.0)
                rec = pf.tile([128, 1], F32, tag="rec")
                nc.vector.reciprocal(out=rec[:], in_=cntm[:])
                scale = pf.tile([128, 1], F32, tag="scale")
                nc.vector.tensor_tensor(out=scale[:], in0=rec[:],
                                        in1=homo_sb[:, t:t + 1],
                                        op=mybir.AluOpType.mult)
                z = po.tile([128, 128], BF16, tag="z")
                nc.vector.memset(z[:, 65:128], 0.0)
                nc.scalar.mul(z[:, 0:64], er[:, 0:64], scale[:])
                nc.scalar.copy(out=z[:, 64:65], in_=homo_sb[:, t:t + 1])
                nc.scalar.dma_start(out=zef_d[t * 128:(t + 1) * 128, :], in_=z[:])

            # phase 1: supertiles of M1 edge tiles share one gather;
            # AllReduce + Ze build per chunk overlap the remaining tiles
            for ci in range(len(cuts) - 1):
                lo, hi = cuts[ci], cuts[ci + 1]
                assert lo % M1 == 0 and hi % M1 == 0
                for sg in range(lo // M1, hi // M1):
                    nbg = int(nbg1[sg]) if cfg.trim_pads else c1 * M1
                    gi = pidx.tile([128, capg1 // 16], I16, tag="gi1")
                    nc.sync.dma_start(out=gi[:], in_=g1_d[sg])
                    oh = pm.tile([128, capg1], BF16, tag="mt1")
                    nc.scalar.dma_start(out=oh[:, 0:nbg * 128],
                                        in_=oh1_d[sg, :, 0:nbg * 128])
                    g = pg.tile([128, c1 * M1, 128], BF16, tag="g1")
                    nc.gpsimd.dma_gather(g[:, 0:nbg, :], xpb_d[:], gi[:],
                                         nbg * 128, nregs[nbg * 128], 128,
                                         single_packet=False,
                                         queue_num=next_q())
                    for u in range(M1):
                        s = sg * M1 + u
                        n1 = int(nb1[s]) if cfg.trim_pads else c1
                        b0 = int(boff1[sg, u]) if cfg.trim_pads else u * c1
                        ps = pp.tile([128, 65], F32, tag="ps1")
                        for j in range(n1):
                            nc.tensor.matmul(
                                ps[:],
                                lhsT=oh[:, (b0 + j) * 128:(b0 + j + 1) * 128],
                                rhs=g[:, b0 + j, 0:65],
                                start=(j == 0), stop=(j == n1 - 1))
                        acc = po.tile([128, 65], BF16, tag="acc1")
                        nc.vector.tensor_copy(out=acc[:], in_=ps[:])
                        nc.sync.dma_start(out=eacc_d[s * 128:(s + 1) * 128, :], in_=acc[:])
                nc.gpsimd.collective_compute(
                    "AllReduce", mybir.AluOpType.add,
                    replica_groups=[list(range(cfg.n_cores))],
                    ins=[eacc_d[lo * 128:hi * 128, :]],
                    outs=[ered_d[lo * 128:hi * 128, :]],
                )
                for t in range(lo, hi):
                    ze_build(t)

            # phase 2: node-tile accumulation + finalize
            for s in range(cfg.ntiles):
                    gi = pidx.tile([128, cfg.cap2 // 16], I16, tag="gi2")
                    nc.sync.dma_start(out=gi[:], in_=g2_d[s])
                    n2 = (nb2[s] if cfg.trim_pads else c2)
                    oh = pm.tile([128, c2 * 128], BF16, tag="mt2")
                    nc.scalar.dma_start(out=oh[:, 0:n2 * 128],
                                        in_=oh2_d[s, :, 0:n2 * 128])
                    g = pg.tile([128, c2, 128], BF16, tag="g2")
                    nc.gpsimd.dma_gather(g[:, 0:n2, :], zef_d[:], gi[:],
                                         n2 * 128, nregs[n2 * 128], 128,
                                         single_packet=False, queue_num=next_q())
                    ps = pp.tile([128, 65], F32, tag="ps2")
                    for j in range(n2):
                        nc.tensor.matmul(ps[:, 0:65], lhsT=oh[:, j * 128:(j + 1) * 128],
                                         rhs=g[:, j, 0:65],
                                         start=(j == 0), stop=(j == n2 - 1))
                    attm = pf.tile([128, 1], F32, tag="attm")
                    nc.vector.tensor_scalar_max(out=attm[:], in0=ps[:, 64:65], scalar1=1e-30)
                    arec = pf.tile([128, 1], F32, tag="arec")
                    nc.vector.reciprocal(out=arec[:], in_=attm[:])
                    xp_sb = pf.tile([128, 64], F32, tag="xpl")
                    nc.scalar.dma_start(out=xp_sb[:], in_=xp_d[s * 128:(s + 1) * 128, :])
                    o = pf.tile([128, 64], F32, tag="o")
                    nc.scalar.mul(o[:], ps[:, 0:64], arec[:])
                    nc.vector.tensor_tensor(out=o[:], in0=o[:], in1=xp_sb[:],
                                            op=mybir.AluOpType.add)
                    sq = pf.tile([128, 64], F32, tag="sq")
                    rs = pf.tile([128, 1], F32, tag="rs")
                    nc.scalar.activation(out=sq[:], in_=o[:],
                                         func=mybir.ActivationFunctionType.Square,
                                         accum_out=rs[:])
                    rn = pf.tile([128, 1], F32, tag="rn")
                    nc.scalar.sqrt(out=rn[:], in_=rs[:])
                    rnm = pf.tile([128, 1], F32, tag="rnm")
                    nc.vector.tensor_scalar_max(out=rnm[:], in0=rn[:], scalar1=1e-30)
                    rrec = pf.tile([128, 1], F32, tag="rrec")
                    nc.vector.reciprocal(out=rrec[:], in_=rnm[:])
                    ot = po.tile([128, 64], F32, tag="ot")
                    nc.scalar.mul(ot[:], o[:], rrec[:])
                    nc.sync.dma_start(out=out_d[s * 128:(s + 1) * 128, :], in_=ot[:])

    nc.compile()
    return nc


_NC_CACHE = {}
_LAST_RESULT = None


def kernel(**inputs) -> np.ndarray:
    """Full inputs in, full output out. Shards across 8 NeuronCores internally."""
    X = np.asarray(inputs["X"], dtype=np.float32)
    W = np.asarray(inputs["W"], dtype=np.float32)
    homo = np.asarray(inputs["homo"], dtype=np.float32)
    vertex = np.asarray(inputs["vertex"])
    edges = np.asarray(inputs["edges"])
    cfg = Cfg(
        nqueues=int(os.environ.get("KERNEL_NQ", "4")),
        ar_chunks=int(os.environ.get("KERNEL_ARC", "4")),
        grp=int(os.environ.get("KERNEL_GRP", "1")),
        m1=int(os.environ.get("KERNEL_M1", "4")),
        trim_pads=os.environ.get("KERNEL_TRIM", "1") == "1",
    )
    assert X.shape == (cfg.N, 64) and homo.shape == (cfg.E,)

    in_maps, nb1, nb2, _, _ = prep_inputs(cfg, X, W, homo, vertex, edges)
    key = (cfg, nb1, nb2)
    if key not in _NC_CACHE:
        _NC_CACHE[key] = build_nc(cfg, nb1, nb2)
    nc = _NC_CACHE[key]
    res = bass_utils.run_bass_kernel_spmd(
        nc, in_maps, core_ids=list(range(cfg.n_cores)),
        trace=bool(os.environ.get("KERNEL_TRACE")))
    global _LAST_RESULT
    _LAST_RESULT = res
    out = np.concatenate(
        [res.results[k]["out"][:cfg.npc] for k in range(cfg.n_cores)], axis=0)
    return out.astype(np.float32)


# revision 27
# speedup vs baseline: 1.0019x; 1.0019x over previous
"""HyperGNN message-passing kernel (nn_Conv_13778255086166) for 8 TRN2 NeuronCores.

Reference computation:
    Xp    = X @ W                                   [N, 64]
    Xe_s  = segment_sum(Xp[vertex], edges, E);  cnt = segment_sum(1, edges, E)
    Ze    = (homo / max(cnt,1)) * Xe_s              [E, 64]   (mean aggregation * homo)
    att_s = segment_sum(homo[edges], vertex, N)
    Xv    = segment_sum(Ze[edges], vertex, N) / att_s
    out   = row_l2_normalize(Xp + Xv)

Distribution (graph parallelism): incidences sharded by vertex range; core k
owns nodes [k*12500, (k+1)*12500).  Per core:

  phase 0: Xp = X_local @ W -> fp32 XpD [npcp, 64] (finalize) and a bf16
           gather table XpB [npcp, 128] (cols 0:64 features, col 64 = count
           marker 1.0, pad rows 0.0, rest zero).
  phase 1: per 128-edge tile, dma_gather the XpB rows of the (host-sorted,
           tail-padded) incidence slots and accumulate per 128-slot block
           onehot.T @ g[:, j, 0:65] into PSUM -> [sums | cnt].  The one-hot
           selection matrices are HOST-PRECOMPUTED (pure index formatting)
           and streamed in as bf16 — nothing builds them on-device, which
           keeps the Vector engine out of the Q7 descriptor-generation
           windows (they interfere on the shared SBUF port).  Gather pads
           carry index -1 (the Q7 ucode trims trailing negatives; num_idxs
           per tile = shared max block count so the trimmed count matches
           the decode-side ring accounting on every core).  Gathers
           round-robin over 4 SWDGE queues; their num_idxs registers are
           preloaded once so the GPSIMD queue holds gathers back-to-back.
  AllReduce(Eacc bf16) in chunks; each chunk's reduce and Ze build overlap
           the remaining phase-1 tiles.
  Ze build: ZeF[:, 0:64] = Ered * homo / max(cnt, 1); ZeF[:, 64] = homo.
  phase 2: per 128-node tile, same gather + one-hot matmul against ZeF ->
           PSUM [sum Ze | att_sum]; finalize Xv = S / max(att, eps);
           out = (Xp + Xv) / rownorm (Square/rowsum/scales on the idle ACT
           engine).

All arithmetic (matmul, all segment sums, normalizations) runs on device.
The host only reorganizes the incidence lists (shard by vertex range, order
by segment, pad to fixed per-tile capacity) and formats index/selection
tensors — schedule/layout preparation, not computation.
"""

import os
from dataclasses import dataclass

import ml_dtypes
import numpy as np

import concourse.bacc as bacc
import concourse.mybir as mybir
import concourse.tile as tile
from concourse import bass_utils

F32 = mybir.dt.float32
BF16 = mybir.dt.bfloat16
I16 = mybir.dt.int16
BF = ml_dtypes.bfloat16


@dataclass(frozen=True)
class Cfg:
    n_cores: int = 8
    N: int = 100000
    E: int = 25000
    cap1: int = 1536   # incidence slots per 128-edge tile per core (mult of 128)
    cap2: int = 2944   # incidence slots per 128-node tile per core (mult of 128)
    nqueues: int = 4   # SWDGE queues used for dma_gather round-robin
    ar_chunks: int = 4  # AllReduce chunks overlapped with phase 1
    grp: int = 1       # gathers hoisted per group (>1 corrupts: decode race)
    m1: int = 4        # phase-1 edge tiles merged per dma_gather
    trim_pads: bool = True  # -1 gather pads (Q7 trims trailing negatives)

    @property
    def npc(self):
        assert self.N % self.n_cores == 0
        return self.N // self.n_cores

    @property
    def npcp(self):  # padded, with at least one spare zero row
        return (self.npc + 1 + 127) // 128 * 128

    @property
    def ntiles(self):
        return self.npcp // 128

    @property
    def ep(self):
        return (self.E + 1 + 127) // 128 * 128

    @property
    def etiles(self):
        return self.ep // 128


def wrap_idx(idx: np.ndarray) -> np.ndarray:
    """int16 index layout for dma_gather: element j at [j%16, j//16],
    replicated across the 8 16-partition groups (one per Q7 cpu)."""
    s = idx.shape[0]
    assert s % 16 == 0
    w = np.ascontiguousarray(idx.astype(np.int16).reshape(-1, 16).T)
    return np.tile(w, (8, 1))


def prep_inputs(cfg: Cfg, X, W, homo, vertex, edges):
    """Host-side shard/sort/pad for all cores (index/layout reorganization
    only).  Returns (in_maps, nb1, nb2) where nb1/nb2 are the per-tile
    128-slot block counts shared across cores (max over cores)."""
    npc, npcp = cfg.npc, cfg.npcp
    vertex = np.asarray(vertex)
    edges = np.asarray(edges)

    def build_core(k):
        sel = (vertex >= k * npc) & (vertex < (k + 1) * npc)
        v_l = (vertex[sel] - k * npc).astype(np.int64)
        e_l = edges[sel].astype(np.int64)
        return v_l, e_l

    shards = [build_core(k) for k in range(cfg.n_cores)]

    def sort_one(seg, other, tiles_n, cap):
        o = np.argsort(seg, kind="stable")
        s, g = seg[o], other[o]
        t_of = s >> 7
        counts = np.bincount(t_of, minlength=tiles_n)
        assert (counts <= cap).all(), (counts.max(), cap)
        return s, g, t_of, counts

    sorted1 = [sort_one(e_l, v_l, cfg.etiles, cfg.cap1) for v_l, e_l in shards]
    sorted2 = [sort_one(v_l, e_l, cfg.ntiles, cfg.cap2) for v_l, e_l in shards]

    # shared per-tile block counts: max over cores, >= 1
    nb1 = np.maximum(1, -(-np.stack([c for _, _, _, c in sorted1]).max(0) // 128))
    nb2 = np.maximum(1, -(-np.stack([c for _, _, _, c in sorted2]).max(0) // 128))

    def build(sorted_sg, nb, cap, pad_gather, M):
        """Supertiles of M consecutive 128-seg tiles share one gather: each
        sub-tile's live span [boff, boff + nb*128) is packed back-to-back so
        the supertile's trimmed index count is sum(nb)*128."""
        s, g, t_of, counts = sorted_sg
        tiles_n = len(counts)
        assert tiles_n % M == 0
        gtiles = tiles_n // M
        capg = cap * M
        starts = np.cumsum(counts) - counts
        rank = np.arange(len(s)) - starts[t_of]
        if not cfg.trim_pads:
            assert M == 1
            boff = np.zeros(tiles_n, np.int64)
            nbg = np.full(gtiles, cap // 128 * M, np.int64)
        else:
            nbr = nb.reshape(gtiles, M)
            boff = (np.cumsum(nbr, 1) - nbr).reshape(tiles_n) * 128
            nbg = nbr.sum(1)
        g_of = t_of // M
        dest = g_of * capg + boff[t_of] + rank
        S = gtiles * capg
        gi = np.full(S, -1, np.int64)
        off = np.full(S, 255, np.int64)
        ar = np.arange(capg)
        live = ar[None, :] < (nbg * 128)[:, None]
        gi[live.ravel()] = pad_gather
        gi[dest] = g
        off[dest] = s & 127
        # one-hot selection matrices [gtiles, 128 slot-partitions, blocks*128]
        oh = (off.reshape(gtiles, capg // 128, 128)[:, :, :, None]
              == np.arange(128)[None, None, None, :])
        oh = np.ascontiguousarray(
            oh.transpose(0, 2, 1, 3).reshape(gtiles, 128, capg)
        ).astype(BF)
        return gi, oh, tuple(int(x) for x in nbg)

    def tilemaj_idx(gi, tiles_n, cap):
        w = np.stack([wrap_idx(gi[t * cap:(t + 1) * cap]) for t in range(tiles_n)])
        return np.ascontiguousarray(w)

    homo_pad = np.zeros(cfg.ep, np.float32)
    homo_pad[:cfg.E] = np.asarray(homo)
    homo_t = np.ascontiguousarray(homo_pad.reshape(cfg.etiles, 128).T)

    mark = (np.arange(npcp).reshape(cfg.ntiles, 128).T < npc).astype(np.float32)
    mark = np.ascontiguousarray(mark)

    in_maps = []
    for k in range(cfg.n_cores):
        g1, oh1, nbg1 = build(sorted1[k], nb1, cfg.cap1, npc, cfg.m1)
        g2, oh2, nbg2 = build(sorted2[k], nb2, cfg.cap2, cfg.E, 1)
        Xt = np.zeros((64, npcp), np.float32)
        Xt[:, :npc] = np.asarray(X)[k * npc:(k + 1) * npc].T
        in_maps.append({
            "Xt": Xt,
            "W": np.asarray(W, dtype=np.float32),
            "homo_t": homo_t,
            "mark": mark,
            "g1": tilemaj_idx(g1, cfg.etiles // cfg.m1, cfg.cap1 * cfg.m1),
            "oh1": oh1,
            "g2": tilemaj_idx(g2, cfg.ntiles, cfg.cap2),
            "oh2": oh2,
        })
    return (in_maps, tuple(int(x) for x in nb1), tuple(int(x) for x in nb2),
            nbg1, nbg2)


def build_nc(cfg: Cfg, nb1, nb2):
    # phase-1 supertile block counts/bases (shared across cores)
    M1 = cfg.m1
    nb1r = np.array(nb1).reshape(-1, M1)
    nbg1 = nb1r.sum(1)
    boff1 = (np.cumsum(nb1r, 1) - nb1r)
    c1 = cfg.cap1 // 128
    c2 = cfg.cap2 // 128
    nc = bacc.Bacc("TRN2", target_bir_lowering=False, debug=False,
                   num_devices=cfg.n_cores, num_swdge_queues=cfg.nqueues)

    xt_d = nc.dram_tensor("Xt", [64, cfg.npcp], F32, kind="ExternalInput")
    w_d = nc.dram_tensor("W", [64, 64], F32, kind="ExternalInput")
    homo_d = nc.dram_tensor("homo_t", [128, cfg.etiles], F32, kind="ExternalInput")
    gt1 = cfg.etiles // cfg.m1
    capg1 = cfg.cap1 * cfg.m1
    g1_d = nc.dram_tensor("g1", [gt1, 128, capg1 // 16], I16, kind="ExternalInput")
    oh1_d = nc.dram_tensor("oh1", [gt1, 128, capg1], BF16, kind="ExternalInput")
    g2_d = nc.dram_tensor("g2", [cfg.ntiles, 128, cfg.cap2 // 16], I16, kind="ExternalInput")
    oh2_d = nc.dram_tensor("oh2", [cfg.ntiles, 128, cfg.cap2], BF16, kind="ExternalInput")
    mark_d = nc.dram_tensor("mark", [128, cfg.ntiles], F32, kind="ExternalInput")
    out_d = nc.dram_tensor("out", [cfg.npcp, 64], F32, kind="ExternalOutput")

    xp_d = nc.dram_tensor("XpD", [cfg.npcp, 64], F32, kind="Internal")
    xpb_d = nc.dram_tensor("XpB", [cfg.npcp, 128], BF16, kind="Internal")
    eacc_d = nc.dram_tensor("EaccD", [cfg.ep, 65], BF16, kind="Internal")
    ered_d = nc.dram_tensor("EredD", [cfg.ep, 65], BF16, kind="Internal", addr_space="Shared")
    zef_d = nc.dram_tensor("ZeFD", [cfg.ep, 128], BF16, kind="Internal")

    qn = [0]

    def next_q():
        q = qn[0]
        qn[0] = (q + 1) % cfg.nqueues
        return q

    # phase-1 AllReduce chunk boundaries (tile index); last chunk smallest so
    # the un-overlapped tail is short
    if cfg.ar_chunks == 4:
        cuts = [0, 64, 112, 160, cfg.etiles]
    elif cfg.ar_chunks == 2:
        cuts = [0, cfg.etiles // 2, cfg.etiles]
    else:
        cuts = [0, cfg.etiles]

    with tile.TileContext(nc) as tc:
        with (
            tc.tile_pool(name="const", bufs=1) as pc,
            tc.tile_pool(name="idx", bufs=8) as pidx,
            tc.tile_pool(name="gather", bufs=6) as pg,
            tc.tile_pool(name="onehot", bufs=4) as pm,
            tc.tile_pool(name="sbout", bufs=4) as po,
            tc.tile_pool(name="fin", bufs=4) as pf,
            tc.tile_pool(name="psum", bufs=2, space="PSUM") as pp,
        ):
            # num_idxs registers, preloaded once per distinct value
            nregs = {}
            for v in sorted({int(n) * 128 for n in (*nbg1, *nb2)}):
                r = nc.gpsimd.alloc_register(f"nidx_{v}")
                nc.gpsimd.reg_mov(r, v)
                nregs[v] = r

            w_sb = pc.tile([64, 64], F32)
            nc.sync.dma_start(out=w_sb[:], in_=w_d[:])
            homo_sb = pc.tile([128, cfg.etiles], F32)
            nc.sync.dma_start(out=homo_sb[:], in_=homo_d[:])
            mark_sb = pc.tile([128, cfg.ntiles], F32)
            nc.sync.dma_start(out=mark_sb[:], in_=mark_d[:])

            # phase 0: Xp = X_local @ W; fp32 table + bf16 gather table
            for t in range(cfg.ntiles):
                xtt = pf.tile([64, 128], F32, tag="xt0")
                nc.sync.dma_start(out=xtt[:], in_=xt_d[:, t * 128:(t + 1) * 128])
                ps = pp.tile([128, 64], F32, tag="ps0")
                nc.tensor.matmul(ps[:], lhsT=xtt[:],
                                 rhs=w_sb[:], start=True, stop=True)
                xp_sb = po.tile([128, 64], F32, tag="xp0")
                nc.vector.tensor_copy(out=xp_sb[:], in_=ps[:])
                nc.scalar.dma_start(out=xp_d[t * 128:(t + 1) * 128, :], in_=xp_sb[:])
                xb = po.tile([128, 128], BF16, tag="xb0")
                nc.vector.memset(xb[:, 65:128], 0.0)
                # count marker: 1.0 for real rows, 0.0 for pad/zero rows
                nc.vector.tensor_copy(out=xb[:, 64:65], in_=mark_sb[:, t:t + 1])
                nc.vector.tensor_copy(out=xb[:, 0:64], in_=ps[:])
                nc.sync.dma_start(out=xpb_d[t * 128:(t + 1) * 128, :], in_=xb[:])

            # Ze build for one edge tile (runs as soon as its AR chunk lands)
            def ze_build(t):
                er = pf.tile([128, 65], BF16, tag="er")
                nc.scalar.dma_start(out=er[:], in_=ered_d[t * 128:(t + 1) * 128, :])
                cntm = pf.tile([128, 1], F32, tag="cntm")
                nc.vector.tensor_scalar_max(out=cntm[:], in0=er[:, 64:65], scalar1=1.0)
                rec = pf.tile([128, 1], F32, tag="rec")
                nc.vector.reciprocal(out=rec[:], in_=cntm[:])
                scale = pf.tile([128, 1], F32, tag="scale")
                nc.vector.tensor_tensor(out=scale[:], in0=rec[:],
                                        in1=homo_sb[:, t:t + 1],
                                        op=mybir.AluOpType.mult)
                z = po.tile([128, 128], BF16, tag="z")
                nc.vector.memset(z[:, 65:128], 0.0)
                nc.scalar.mul(z[:, 0:64], er[:, 0:64], scale[:])
                nc.scalar.copy(out=z[:, 64:65], in_=homo_sb[:, t:t + 1])
                nc.scalar.dma_start(out=zef_d[t * 128:(t + 1) * 128, :], in_=z[:])

            # phase 1: supertiles of M1 edge tiles share one gather.
            # Each chunk's AllReduce is emitted a few supertiles into the
            # NEXT chunk so its queue-head wait (on the chunk's eacc writes)
            # is already satisfied and never stalls gather dispatch; the Ze
            # builds follow the AllReduce.
            DELAY = 3

            def emit_ar(lo, hi):
                nc.gpsimd.collective_compute(
                    "AllReduce", mybir.AluOpType.add,
                    replica_groups=[list(range(cfg.n_cores))],
                    ins=[eacc_d[lo * 128:hi * 128, :]],
                    outs=[ered_d[lo * 128:hi * 128, :]],
                )
                for t in range(lo, hi):
                    ze_build(t)

            pending = []
            for ci in range(len(cuts) - 1):
                lo, hi = cuts[ci], cuts[ci + 1]
                assert lo % M1 == 0 and hi % M1 == 0
                for sg in range(lo // M1, hi // M1):
                    if pending and sg >= pending[0][0]:
                        _, plo, phi = pending.pop(0)
                        emit_ar(plo, phi)
                    nbg = int(nbg1[sg]) if cfg.trim_pads else c1 * M1
                    gi = pidx.tile([128, capg1 // 16], I16, tag="gi1")
                    nc.sync.dma_start(out=gi[:], in_=g1_d[sg])
                    oh = pm.tile([128, capg1], BF16, tag="mt1")
                    nc.scalar.dma_start(out=oh[:, 0:nbg * 128],
                                        in_=oh1_d[sg, :, 0:nbg * 128])
                    g = pg.tile([128, c1 * M1, 128], BF16, tag="g1")
                    nc.gpsimd.dma_gather(g[:, 0:nbg, :], xpb_d[:], gi[:],
                                         nbg * 128, nregs[nbg * 128], 128,
                                         single_packet=False,
                                         queue_num=next_q())
                    for u in range(M1):
                        s = sg * M1 + u
                        n1 = int(nb1[s]) if cfg.trim_pads else c1
                        b0 = int(boff1[sg, u]) if cfg.trim_pads else u * c1
                        ps = pp.tile([128, 65], F32, tag="ps1")
                        for j in range(n1):
                            nc.tensor.matmul(
                                ps[:],
                                lhsT=oh[:, (b0 + j) * 128:(b0 + j + 1) * 128],
                                rhs=g[:, b0 + j, 0:65],
                                start=(j == 0), stop=(j == n1 - 1))
                        acc = po.tile([128, 65], BF16, tag="acc1")
                        nc.vector.tensor_copy(out=acc[:], in_=ps[:])
                        nc.sync.dma_start(out=eacc_d[s * 128:(s + 1) * 128, :], in_=acc[:])
                pending.append((hi // M1 + DELAY, lo, hi))
            for _, plo, phi in pending:
                emit_ar(plo, phi)

            # phase 2: node-tile accumulation + finalize
            for s in range(cfg.ntiles):
                    gi = pidx.tile([128, cfg.cap2 // 16], I16, tag="gi2")
                    nc.sync.dma_start(out=gi[:], in_=g2_d[s])
                    n2 = (nb2[s] if cfg.trim_pads else c2)
                    oh = pm.tile([128, c2 * 128], BF16, tag="mt2")
                    nc.scalar.dma_start(out=oh[:, 0:n2 * 128],
                                        in_=oh2_d[s, :, 0:n2 * 128])
                    g = pg.tile([128, c2, 128], BF16, tag="g2")
                    nc.gpsimd.dma_gather(g[:, 0:n2, :], zef_d[:], gi[:],
                                         n2 * 128, nregs[n2 * 128], 128,
                                         single_packet=False, queue_num=next_q())
                    ps = pp.tile([128, 65], F32, tag="ps2")
                    for j in range(n2):
                        nc.tensor.matmul(ps[:, 0:65], lhsT=oh[:, j * 128:(j + 1) * 128],
                                         rhs=g[:, j, 0:65],
                                         start=(j == 0), stop=(j == n2 - 1))
                    attm = pf.tile([128, 1], F32, tag="attm")
                    nc.vector.tensor_scalar_max(out=attm[:], in0=ps[:, 64:65], scalar1=1e-30)
                    arec = pf.tile([128, 1], F32, tag="arec")
                    nc.vector.reciprocal(out=arec[:], in_=attm[:])
                    xp_sb = pf.tile([128, 64], F32, tag="xpl")
                    nc.scalar.dma_start(out=xp_sb[:], in_=xp_d[s * 128:(s + 1) * 128, :])
                    o = pf.tile([128, 64], F32, tag="o")
                    nc.scalar.mul(o[:], ps[:, 0:64], arec[:])
                    nc.vector.tensor_tensor(out=o[:], in0=o[:], in1=xp_sb[:],
                                            op=mybir.AluOpType.add)
                    sq = pf.tile([128, 64], F32, tag="sq")
                    rs = pf.tile([128, 1], F32, tag="rs")
                    nc.scalar.activation(out=sq[:], in_=o[:],
                                         func=mybir.ActivationFunctionType.Square,
                                         accum_out=rs[:])
                    rn = pf.tile([128, 1], F32, tag="rn")
                    nc.scalar.sqrt(out=rn[:], in_=rs[:])
                    rnm = pf.tile([128, 1], F32, tag="rnm")
                    nc.vector.tensor_scalar_max(out=rnm[:], in0=rn[:], scalar1=1e-30)
                    rrec = pf.tile([128, 1], F32, tag="rrec")
                    nc.vector.reciprocal(out=rrec[:], in_=rnm[:])
                    ot = po.tile([128, 64], F32, tag="ot")
                    nc.scalar.mul(ot[:], o[:], rrec[:])
                    nc.sync.dma_start(out=out_d[s * 128:(s + 1) * 128, :], in_=ot[:])

    nc.compile()
    return nc


_NC_CACHE = {}
_LAST_RESULT = None


def kernel(**inputs) -> np.ndarray:
    """Full inputs in, full output out. Shards across 8 NeuronCores internally."""
    X = np.asarray(inputs["X"], dtype=np.float32)
    W = np.asarray(inputs["W"], dtype=np.float32)
    homo = np.asarray(inputs["homo"], dtype=np.float32)
    vertex = np.asarray(inputs["vertex"])
    edges = np.asarray(inputs["edges"])
    cfg = Cfg(
        nqueues=int(os.environ.get("KERNEL_NQ", "4")),
        ar_chunks=int(os.environ.get("KERNEL_ARC", "4")),
        grp=int(os.environ.get("KERNEL_GRP", "1")),
        m1=int(os.environ.get("KERNEL_M1", "4")),
        trim_pads=os.environ.get("KERNEL_TRIM", "1") == "1",
    )
    assert X.shape == (cfg.N, 64) and homo.shape == (cfg.E,)

    in_maps, nb1, nb2, _, _ = prep_inputs(cfg, X, W, homo, vertex, edges)
    key = (cfg, nb1, nb2)
    if key not in _NC_CACHE:
        _NC_CACHE[key] = build_nc(cfg, nb1, nb2)
    nc = _NC_CACHE[key]
    res = bass_utils.run_bass_kernel_spmd(
        nc, in_maps, core_ids=list(range(cfg.n_cores)),
        trace=bool(os.environ.get("KERNEL_TRACE")))
    global _LAST_RESULT
    _LAST_RESULT = res
    out = np.concatenate(
        [res.results[k]["out"][:cfg.npc] for k in range(cfg.n_cores)], axis=0)
    return out.astype(np.float32)
